# revision 5
# baseline (speedup 1.0000x reference)
"""Trainium2 Bass kernel for GQA attention block (B=1, S=2048, DIM=4096,
32 q heads / 8 kv heads, head_dim 128, RoPE, causal, fused QKV + out proj).

Sharding: tensor-parallel over heads across 8 cores. Core i computes
q heads 4i..4i+3 and kv head i (one full GQA group), plus the wo
contribution of its 512 output columns; host sums the 8 partial outputs.

All heavy matmuls run in bf16 (inputs quantized host-side; rel err vs
fp32 reference ~3e-3 << 2e-2 gate). bf16 halves SBUF/PSUM/DMA traffic
vs f32r, which is what limited the f32r version (~1.3-1.7x stretch of
every matmul from SBUF port contention).
"""
import numpy as np
import ml_dtypes

import concourse.bass as bass
import concourse.mybir as mybir
import concourse.tile as tile
from concourse import bacc
from concourse.bass_utils import run_bass_kernel_spmd
from concourse.masks import make_identity

F32 = mybir.dt.float32
F32R = mybir.dt.float32r
BF16 = mybir.dt.bfloat16
AF = mybir.ActivationFunctionType

B, S, DIM = 1, 2048, 4096
N_HEADS, N_KV_HEADS = 32, 8
HD = DIM // N_HEADS              # 128
N_CORES = 8
QH = N_HEADS // N_CORES          # 4 q heads per core
OC = QH * HD + 2 * HD            # 768 per-core qkv output columns
NS = S // 128                    # 16 s-blocks
ND = DIM // 128                  # 32 d-blocks
XSUB = 8                         # d-blocks per x sub-tile in phase 1
NXS = ND // XSUB                 # 4 x sub-tiles per s-block
WSUB = 4                         # d-blocks per w load chunk
STILE = 512                      # s-tile width in phase 2/3
NST = S // STILE                 # 4 s-tiles
NDC = DIM // 512                 # 8 output column chunks
SCALE = 1.0 / float(np.sqrt(HD))
MASK_NEG = -1.0e5


def _build_nc():
    nc = bacc.Bacc("TRN2", target_bir_lowering=False, debug=False)

    # host-pre-tiled inputs (see _prep_in_maps for layouts)
    xt = nc.dram_tensor("xt", [NS, NXS, 128, XSUB, 128], BF16,
                        kind="ExternalInput").ap()
    wt = nc.dram_tensor("wt", [128, ND, OC], BF16, kind="ExternalInput").ap()
    wot = nc.dram_tensor("wot", [128, NDC, QH, 512], BF16,
                         kind="ExternalInput").ap()
    cos5 = nc.dram_tensor("cos5", [S, 5 * 64], F32, kind="ExternalInput").ap()
    sin5 = nc.dram_tensor("sin5", [S, 5 * 64], F32, kind="ExternalInput").ap()
    # [128, 128] additive triangle mask for the diagonal 128-block
    cmask = nc.dram_tensor("cmask", [128, 128], F32, kind="ExternalInput").ap()
    y = nc.dram_tensor("y", [S, DIM], BF16, kind="ExternalOutput").ap()

    with tile.TileContext(nc) as tc:
        _emit(tc, nc, xt, wt, wot, cos5, sin5, cmask, y)
    nc.compile()
    return nc


def _emit(tc, nc, xt, wt, wot, cos5, sin5, cmask, y):
    import contextlib

    with contextlib.ExitStack() as ctx:
        # ---------- long-lived tiles ----------
        keep = ctx.enter_context(tc.tile_pool(name="keep", bufs=1))
        # QT_all[:, h, :]: per-head roped Q transposed [d, s]; h=QH is roped K
        QT_all = keep.tile([128, QH + 1, S], BF16)
        V_all = keep.tile([128, NS, HD], BF16)          # V blocks [t, d]
        OT_all = keep.tile([128, QH, S], BF16)          # attn out transposed
        wo_sb = keep.tile([128, NDC, QH, 512], BF16)    # full wo slice (4MB)
        ident = keep.tile([128, 128], BF16)
        make_identity(nc, ident)
        ones_f = keep.tile([128, 128], F32)
        nc.vector.memset(ones_f, 1.0)
        ones_r = keep.tile([128, 128], F32R)
        nc.vector.tensor_copy(ones_r, ones_f)
        tri_mask = keep.tile([128, 128], F32)
        nc.gpsimd.dma_start(tri_mask, cmask)

        # ---------- phase 1: qkv projection + RoPE + transposes ----------
        with (
            tc.tile_pool(name="p1w", bufs=1) as p1w,
            tc.tile_pool(name="p1x", bufs=2) as p1x,
            tc.tile_pool(name="p1t", bufs=1) as p1t,
            tc.tile_pool(name="p1ps", bufs=1, space="PSUM") as p1ps,
        ):
            # first x sub-tile before the w bulk so PE can start ASAP
            x_first = p1x.tile([128, XSUB, 128], BF16, tag="x")
            nc.scalar.dma_start(x_first, xt[0, 0])
            # w chunked so the first matmuls can start after the first chunk
            w_sb = p1w.tile([128, ND, OC], BF16)
            for ci in range(ND // WSUB):
                nc.sync.dma_start(
                    w_sb[:, WSUB * ci:WSUB * (ci + 1), :],
                    wt[:, WSUB * ci:WSUB * (ci + 1), :],
                )
            # prefetch the whole wo slice during phase 1 (sync queue drains
            # after the w chunks; transfer hides under phase-1 compute)
            nc.sync.dma_start(wo_sb, wot)

            # s-blocks in groups of GRP: w-chunk-major matmul order inside a
            # group so PE consumption tracks the streaming w arrival.
            GRP = 4
            groups = [list(range(g, min(g + GRP, NS))) for g in range(0, NS, GRP)]
            for group in groups:
                ps_qs = {}
                ps_kvs = {}
                x_tiles = {}
                for sb in group:
                    ps_qs[sb] = p1ps.tile([128, 512], F32, tag=f"psq{sb % GRP}", name=f"psq{sb}")
                    ps_kvs[sb] = p1ps.tile([128, 256], F32, tag=f"pskv{sb % GRP}", name=f"pskv{sb}")
                for xs in range(NXS):
                    for sb in group:
                        if sb == 0 and xs == 0:
                            x_tiles[sb] = x_first
                        else:
                            x_tiles[sb] = p1x.tile(
                                [128, XSUB, 128], BF16, tag=f"x{sb % GRP}",
                                name=f"x{sb}_{xs}")
                            nc.scalar.dma_start(x_tiles[sb], xt[sb, xs])
                    for sb in group:
                        x_sb = x_tiles[sb]
                        for dbi in range(XSUB):
                            db = XSUB * xs + dbi
                            nc.tensor.matmul(
                                ps_qs[sb], lhsT=x_sb[:, dbi, :],
                                rhs=w_sb[:, db, 0:512],
                                start=(db == 0), stop=(db == ND - 1),
                            )
                            nc.tensor.matmul(
                                ps_kvs[sb], lhsT=x_sb[:, dbi, :],
                                rhs=w_sb[:, db, 512:768],
                                start=(db == 0), stop=(db == ND - 1),
                            )
                for sb in group:
                    _rope_and_transpose(
                        tc, nc, p1t, p1ps, cos5, sin5, sb,
                        ps_qs[sb], ps_kvs[sb], QT_all, V_all, ident)

        _emit_attn(tc, nc, ctx, (QT_all, V_all, OT_all, wo_sb, ident, ones_r,
                                 tri_mask), y)


def _rope_and_transpose(tc, nc, p1t, p1ps, cos5, sin5, sb, ps_q, ps_kv,
                        QT_all, V_all, ident):
    # RoPE (q: 4 heads = 512 cols; k: 128 cols), f32 math
    cos_t = p1t.tile([128, 320], F32, tag="cos")
    sin_t = p1t.tile([128, 320], F32, tag="sin")
    nc.gpsimd.dma_start(cos_t, cos5[128 * sb:128 * (sb + 1), :])
    nc.gpsimd.dma_start(sin_t, sin5[128 * sb:128 * (sb + 1), :])

    qk_roped = p1t.tile([128, 640], F32, tag="qkr")
    for part, ps_src, wid in (("q", ps_q, 512), ("k", ps_kv, 128)):
        nf = wid // 2
        off = 0 if part == "q" else 512
        pe = ps_src[:, 0:wid:2]
        po = ps_src[:, 1:wid:2]
        c = cos_t[:, 0:nf]
        sn = sin_t[:, 0:nf]
        t1 = p1t.tile([128, 256], F32, tag="t1")
        t2 = p1t.tile([128, 256], F32, tag="t2")
        nc.vector.tensor_mul(t1[:, 0:nf], pe, c)
        nc.vector.tensor_mul(t2[:, 0:nf], po, sn)
        nc.vector.tensor_sub(
            qk_roped[:, off + 0:off + wid:2], t1[:, 0:nf], t2[:, 0:nf])
        t3 = p1t.tile([128, 256], F32, tag="t3")
        t4 = p1t.tile([128, 256], F32, tag="t4")
        nc.vector.tensor_mul(t3[:, 0:nf], pe, sn)
        nc.vector.tensor_mul(t4[:, 0:nf], po, c)
        nc.vector.tensor_add(
            qk_roped[:, off + 1:off + wid:2], t3[:, 0:nf], t4[:, 0:nf])

    # bf16 copy of the roped q/k for the PE transposes
    qk_bf = p1t.tile([128, 640], BF16, tag="qkb")
    nc.vector.tensor_copy(qk_bf, qk_roped)

    # V block: natural [t, d] (cast f32 psum -> bf16)
    nc.vector.tensor_copy(V_all[:, sb, :], ps_kv[:, 128:256])

    # transpose roped q/k head-slices into QT_all
    for h in range(QH + 1):
        # borrow kv accumulator slots (pool-tag reuse; tile's WAR
        # tracking orders this after the rope/V reads)
        tag = f"psq{sb % 4}" if h % 2 == 0 else f"pskv{sb % 4}"
        ps_t = p1ps.tile([128, 128], BF16, tag=tag, name=f"pst{sb}_{h}")
        nc.tensor.transpose(ps_t, qk_bf[:, 128 * h:128 * (h + 1)], ident)
        nc.vector.tensor_copy(QT_all[:, h, 128 * sb:128 * (sb + 1)], ps_t)


def _emit_attn(tc, nc, ctx, keep_tiles, y):
    (QT_all, V_all, OT_all, wo_sb, ident, ones_r, tri_mask) = keep_tiles
    # ---------- phases 2+3 interleaved per 512-wide s-tile ----------
    with (
        tc.tile_pool(name="p2et", bufs=2) as p2et,
        tc.tile_pool(name="p2t", bufs=4) as p2t,
        tc.tile_pool(name="p2ps", bufs=3, space="PSUM") as p2ps,
        tc.tile_pool(name="p2acc", bufs=2, space="PSUM") as p2acc,
        tc.tile_pool(name="p3y", bufs=6) as p3y,
        tc.tile_pool(name="p3ps", bufs=3, space="PSUM") as p3ps,
    ):
        for st in range(NST):
            nj = 4 * st + 4          # number of t-blocks
            s0 = STILE * st
            for h in range(QH):
                ET = p2et.tile([128, NS, STILE], BF16, tag="et")
                acc = p2t.tile([128, STILE], F32R, tag="acc")
                ps_av = p2acc.tile([128, STILE], F32, tag="av")
                for j in range(nj):
                    k = j - (nj - 4)
                    # diagonal blocks: shrink to the exact causal span
                    m = max(k, 0)
                    off = 128 * m
                    wid = STILE - off
                    ps_st = p2ps.tile([128, STILE], F32, tag="st",
                                      name=f"pst{st}_{h}_{j}")
                    nc.tensor.matmul(
                        ps_st[:, 0:wid],
                        lhsT=QT_all[:, QH, 128 * j:128 * (j + 1)],
                        rhs=QT_all[:, h, s0 + off:s0 + STILE],
                        start=True, stop=True,
                    )
                    if k >= 0:
                        # only the 128-wide diagonal sub-block needs masking
                        nc.vector.tensor_add(
                            ps_st[:, 0:128], ps_st[:, 0:128], tri_mask)
                    nc.scalar.activation(
                        ET[:, j, 0:wid], ps_st[:, 0:wid], AF.Exp, scale=SCALE)
                    nc.tensor.matmul(
                        ps_av[:, off:STILE], lhsT=V_all[:, j, :],
                        rhs=ET[:, j, 0:wid],
                        start=(j == 0), stop=(j == nj - 1),
                        skip_group_check=True,
                    )
                    # denominator: running elementwise sum over t-blocks on
                    # the (otherwise idle) Pool engine
                    if j == 0:
                        nc.gpsimd.tensor_copy(acc, ET[:, 0, :])
                    else:
                        nc.gpsimd.tensor_add(
                            acc[:, off:STILE], acc[:, off:STILE],
                            ET[:, j, 0:wid])
                # single cross-partition reduce for the denominator
                ps_den = p2ps.tile([128, STILE], F32, tag="st",
                                   name=f"den{st}_{h}")
                nc.tensor.matmul(ps_den, lhsT=ones_r, rhs=acc,
                                 start=True, stop=True)
                den_r = p2t.tile([128, STILE], F32, tag="denr")
                nc.vector.reciprocal_approx_fast(den_r, ps_den)
                ot_f = p2t.tile([128, STILE], F32, tag="otf")
                nc.vector.tensor_mul(ot_f, ps_av, den_r)
                nc.vector.tensor_copy(OT_all[:, h, s0:s0 + STILE], ot_f)

            # ---------- phase 3 for this s-tile ----------
            for dc in range(NDC):
                for sbl in range(4):
                    sb = 4 * st + sbl
                    ps_y = p3ps.tile([128, 512], F32, tag="psy",
                                     name=f"psy{st}_{dc}_{sbl}")
                    for ob in range(QH):
                        nc.tensor.matmul(
                            ps_y,
                            lhsT=OT_all[:, ob, 128 * sb:128 * (sb + 1)],
                            rhs=wo_sb[:, dc, ob, :],
                            start=(ob == 0), stop=(ob == QH - 1),
                        )
                    y_sb = p3y.tile([128, 512], BF16, tag="ysb")
                    if sb % 2 == 0:
                        nc.vector.tensor_copy(y_sb, ps_y)
                    else:
                        nc.scalar.copy(y_sb, ps_y)
                    nc.sync.dma_start(
                        y[128 * sb:128 * (sb + 1), 512 * dc:512 * (dc + 1)],
                        y_sb)


_NC_CACHE = None


def _get_nc():
    global _NC_CACHE
    if _NC_CACHE is None:
        _NC_CACHE = _build_nc()
    return _NC_CACHE


def _prep_in_maps(x, freqs_cos, freqs_sin, wqkv, wo):
    bf = ml_dtypes.bfloat16
    xT = x.reshape(S, DIM).T.astype(bf)                        # [DIM, S]
    # xt[sb, xs, p, n, s] = xT[128*(XSUB*xs+n)+p, 128*sb+s]
    xt = np.ascontiguousarray(
        xT.reshape(NXS, XSUB, 128, NS, 128).transpose(3, 0, 2, 1, 4))
    cos5 = np.ascontiguousarray(np.tile(freqs_cos, (1, 5)))    # [S, 320]
    sin5 = np.ascontiguousarray(np.tile(freqs_sin, (1, 5)))

    # additive triangle mask for the diagonal 128x128 sub-block:
    # row t', col c valid iff c >= t'
    tl = np.arange(128)[:, None]
    cl = np.arange(128)[None, :]
    cm = np.where(cl >= tl, 0.0, MASK_NEG).astype(np.float32)
    cm = np.ascontiguousarray(cm)

    in_maps = []
    for i in range(N_CORES):
        wq = wqkv[QH * HD * i: QH * HD * (i + 1)]               # [512, DIM]
        wk = wqkv[N_HEADS * HD + HD * i: N_HEADS * HD + HD * (i + 1)]
        wv = wqkv[N_HEADS * HD + N_KV_HEADS * HD + HD * i:
                  N_HEADS * HD + N_KV_HEADS * HD + HD * (i + 1)]
        wT = np.concatenate([wq, wk, wv], axis=0).T.astype(bf)  # [DIM, 768]
        # wt[p, db, o] = wT[128*db+p, o]
        wt = np.ascontiguousarray(wT.reshape(ND, 128, OC).transpose(1, 0, 2))
        woT = wo[:, QH * HD * i: QH * HD * (i + 1)].T.astype(bf)  # [512, DIM]
        # wot[p, dc, ob, j] = woT[128*ob+p, 512*dc+j]
        wot = np.ascontiguousarray(
            woT.reshape(QH, 128, NDC, 512).transpose(1, 2, 0, 3))
        in_maps.append({
            "xt": xt, "wt": wt, "wot": wot,
            "cos5": cos5, "sin5": sin5, "cmask": cm,
        })
    return in_maps


def kernel(x, freqs_cos, freqs_sin, mask, wqkv, wo, _want_trace=False):
    x = np.asarray(x, np.float32)
    freqs_cos = np.asarray(freqs_cos, np.float32)
    freqs_sin = np.asarray(freqs_sin, np.float32)
    wqkv = np.asarray(wqkv, np.float32)
    wo = np.asarray(wo, np.float32)

    nc = _get_nc()
    in_maps = _prep_in_maps(x, freqs_cos, freqs_sin, wqkv, wo)
    res = run_bass_kernel_spmd(
        nc, in_maps, core_ids=list(range(N_CORES)), trace=_want_trace,
    )
    out = np.zeros((S, DIM), np.float64)
    for r in res.results:
        out += r["y"].astype(np.float64)
    if _want_trace:
        kernel._last_results = res
    return out.astype(np.float32).reshape(B, S, DIM)


# revision 11
# speedup vs baseline: 1.1404x; 1.1404x over previous
"""Trainium2 Bass kernel for GQA attention block (B=1, S=2048, DIM=4096,
32 q heads / 8 kv heads, head_dim 128, RoPE, causal, fused QKV + out proj).

Sharding: tensor-parallel over heads across 8 cores. Core i computes
q heads 4i..4i+3 and kv head i (one full GQA group), plus the wo
contribution of its 512 output columns; host sums the 8 partial outputs.

All heavy matmuls run in bf16 (inputs quantized host-side; rel err vs
fp32 reference ~3e-3 << 2e-2 gate). bf16 halves SBUF/PSUM/DMA traffic
vs f32r, which is what limited the f32r version (~1.3-1.7x stretch of
every matmul from SBUF port contention).
"""
import numpy as np
import ml_dtypes

import concourse.bass as bass
import concourse.mybir as mybir
import concourse.tile as tile
from concourse import bacc
from concourse.bass_utils import run_bass_kernel_spmd
from concourse.masks import make_identity

F32 = mybir.dt.float32
F32R = mybir.dt.float32r
BF16 = mybir.dt.bfloat16
AF = mybir.ActivationFunctionType

B, S, DIM = 1, 2048, 4096
N_HEADS, N_KV_HEADS = 32, 8
HD = DIM // N_HEADS              # 128
N_CORES = 8
QH = N_HEADS // N_CORES          # 4 q heads per core
OC = QH * HD + 2 * HD            # 768 per-core qkv output columns
NS = S // 128                    # 16 s-blocks
ND = DIM // 128                  # 32 d-blocks
XSUB = 8                         # d-blocks per x sub-tile in phase 1
NXS = ND // XSUB                 # 4 x sub-tiles per s-block
WSUB = 4                         # d-blocks per w load chunk
STILE = 512                      # s-tile width in phase 2/3
NST = S // STILE                 # 4 s-tiles
NDC = DIM // 512                 # 8 output column chunks
SCALE = 1.0 / float(np.sqrt(HD))
MASK_NEG = -1.0e5


def _build_nc():
    nc = bacc.Bacc("TRN2", target_bir_lowering=False, debug=False)

    # host-pre-tiled inputs (see _prep_in_maps for layouts)
    xt = nc.dram_tensor("xt", [NS, NXS, 128, XSUB, 128], BF16,
                        kind="ExternalInput").ap()
    wt = nc.dram_tensor("wt", [128, ND, OC], BF16, kind="ExternalInput").ap()
    wot = nc.dram_tensor("wot", [128, NDC, QH, 512], BF16,
                         kind="ExternalInput").ap()
    cos5 = nc.dram_tensor("cos5", [S, 5 * 64], F32, kind="ExternalInput").ap()
    sin5 = nc.dram_tensor("sin5", [S, 5 * 64], F32, kind="ExternalInput").ap()
    # [128, 128] multiplicative 0/1 triangle mask for the diagonal 128-block
    cmask = nc.dram_tensor("cmask", [128, 128], BF16, kind="ExternalInput").ap()
    y = nc.dram_tensor("y", [S, DIM], BF16, kind="ExternalOutput").ap()

    with tile.TileContext(nc) as tc:
        _emit(tc, nc, xt, wt, wot, cos5, sin5, cmask, y)
    nc.compile()
    return nc


def _emit(tc, nc, xt, wt, wot, cos5, sin5, cmask, y):
    import contextlib

    with contextlib.ExitStack() as ctx:
        # ---------- long-lived tiles ----------
        keep = ctx.enter_context(tc.tile_pool(name="keep", bufs=1))
        # QT_all[:, h, :]: per-head roped Q transposed [d, s]; h=QH is roped K
        QT_all = keep.tile([128, QH + 1, S], BF16)
        V_all = keep.tile([128, NS, HD], BF16)          # V blocks [t, d]
        OT_all = keep.tile([128, QH, S], BF16)          # attn out transposed
        wo_sb = keep.tile([128, NDC, QH, 512], BF16)    # full wo slice (4MB)
        ident = keep.tile([128, 128], BF16)
        make_identity(nc, ident)
        ones_f = keep.tile([128, 128], F32)
        nc.vector.memset(ones_f, 1.0)
        ones_r = keep.tile([128, 128], F32R)
        nc.vector.tensor_copy(ones_r, ones_f)
        tri_mask = keep.tile([128, 128], BF16)
        nc.gpsimd.dma_start(tri_mask, cmask)

        # ---------- phase 1: qkv projection + RoPE + transposes ----------
        with (
            tc.tile_pool(name="p1w", bufs=1) as p1w,
            tc.tile_pool(name="p1x", bufs=2) as p1x,
            tc.tile_pool(name="p1t", bufs=1) as p1t,
            tc.tile_pool(name="p1ps", bufs=1, space="PSUM") as p1ps,
        ):
            # first x sub-tile before the w bulk so PE can start ASAP
            x_first = p1x.tile([128, XSUB, 128], BF16, tag="x")
            nc.scalar.dma_start(x_first, xt[0, 0])
            # w chunked so the first matmuls can start after the first chunk
            w_sb = p1w.tile([128, ND, OC], BF16)
            for ci in range(ND // WSUB):
                nc.sync.dma_start(
                    w_sb[:, WSUB * ci:WSUB * (ci + 1), :],
                    wt[:, WSUB * ci:WSUB * (ci + 1), :],
                )
            # prefetch the whole wo slice during phase 1 (sync queue drains
            # after the w chunks; transfer hides under phase-1 compute)
            nc.sync.dma_start(wo_sb, wot)

            # s-blocks in groups of GRP: w-chunk-major matmul order inside a
            # group so PE consumption tracks the streaming w arrival.
            GRP = 4
            groups = [list(range(g, min(g + GRP, NS))) for g in range(0, NS, GRP)]
            for group in groups:
                ps_qs = {}
                ps_kvs = {}
                x_tiles = {}
                for sb in group:
                    ps_qs[sb] = p1ps.tile([128, 512], F32, tag=f"psq{sb % GRP}", name=f"psq{sb}")
                    ps_kvs[sb] = p1ps.tile([128, 256], F32, tag=f"pskv{sb % GRP}", name=f"pskv{sb}")
                for xs in range(NXS):
                    for sb in group:
                        if sb == 0 and xs == 0:
                            x_tiles[sb] = x_first
                        else:
                            x_tiles[sb] = p1x.tile(
                                [128, XSUB, 128], BF16, tag=f"x{sb % GRP}",
                                name=f"x{sb}_{xs}")
                            nc.scalar.dma_start(x_tiles[sb], xt[sb, xs])
                    for sb in group:
                        x_sb = x_tiles[sb]
                        for dbi in range(XSUB):
                            db = XSUB * xs + dbi
                            nc.tensor.matmul(
                                ps_qs[sb], lhsT=x_sb[:, dbi, :],
                                rhs=w_sb[:, db, 0:512],
                                start=(db == 0), stop=(db == ND - 1),
                            )
                            nc.tensor.matmul(
                                ps_kvs[sb], lhsT=x_sb[:, dbi, :],
                                rhs=w_sb[:, db, 512:768],
                                start=(db == 0), stop=(db == ND - 1),
                            )
                for sb in group:
                    _rope_and_transpose(
                        tc, nc, p1t, p1ps, cos5, sin5, sb,
                        ps_qs[sb], ps_kvs[sb], QT_all, V_all, ident)

        _emit_attn(tc, nc, ctx, (QT_all, V_all, OT_all, wo_sb, ident, ones_r,
                                 tri_mask), y)


def _rope_and_transpose(tc, nc, p1t, p1ps, cos5, sin5, sb, ps_q, ps_kv,
                        QT_all, V_all, ident):
    # RoPE (q: 4 heads = 512 cols; k: 128 cols), f32 math
    cos_t = p1t.tile([128, 320], F32, tag="cos")
    sin_t = p1t.tile([128, 320], F32, tag="sin")
    nc.gpsimd.dma_start(cos_t, cos5[128 * sb:128 * (sb + 1), :])
    nc.gpsimd.dma_start(sin_t, sin5[128 * sb:128 * (sb + 1), :])

    qk_roped = p1t.tile([128, 640], F32, tag="qkr")
    for part, ps_src, wid in (("q", ps_q, 512), ("k", ps_kv, 128)):
        nf = wid // 2
        off = 0 if part == "q" else 512
        pe = ps_src[:, 0:wid:2]
        po = ps_src[:, 1:wid:2]
        c = cos_t[:, 0:nf]
        sn = sin_t[:, 0:nf]
        t1 = p1t.tile([128, 256], F32, tag="t1")
        t2 = p1t.tile([128, 256], F32, tag="t2")
        nc.vector.tensor_mul(t1[:, 0:nf], pe, c)
        nc.vector.tensor_mul(t2[:, 0:nf], po, sn)
        nc.vector.tensor_sub(
            qk_roped[:, off + 0:off + wid:2], t1[:, 0:nf], t2[:, 0:nf])
        t3 = p1t.tile([128, 256], F32, tag="t3")
        t4 = p1t.tile([128, 256], F32, tag="t4")
        nc.vector.tensor_mul(t3[:, 0:nf], pe, sn)
        nc.vector.tensor_mul(t4[:, 0:nf], po, c)
        nc.vector.tensor_add(
            qk_roped[:, off + 1:off + wid:2], t3[:, 0:nf], t4[:, 0:nf])

    # bf16 copy of the roped q/k for the PE transposes
    qk_bf = p1t.tile([128, 640], BF16, tag="qkb")
    nc.vector.tensor_copy(qk_bf, qk_roped)

    # V block: natural [t, d] (cast f32 psum -> bf16)
    nc.vector.tensor_copy(V_all[:, sb, :], ps_kv[:, 128:256])

    # transpose roped q/k head-slices into QT_all
    for h in range(QH + 1):
        # borrow kv accumulator slots (pool-tag reuse; tile's WAR
        # tracking orders this after the rope/V reads)
        tag = f"psq{sb % 4}" if h % 2 == 0 else f"pskv{sb % 4}"
        ps_t = p1ps.tile([128, 128], BF16, tag=tag, name=f"pst{sb}_{h}")
        nc.tensor.transpose(ps_t, qk_bf[:, 128 * h:128 * (h + 1)], ident)
        nc.vector.tensor_copy(QT_all[:, h, 128 * sb:128 * (sb + 1)], ps_t)


def _emit_attn(tc, nc, ctx, keep_tiles, y):
    (QT_all, V_all, OT_all, wo_sb, ident, ones_r, tri_mask) = keep_tiles
    # ---------- phases 2+3 interleaved per 512-wide s-tile ----------
    with (
        tc.tile_pool(name="p2et", bufs=2) as p2et,
        tc.tile_pool(name="p2t", bufs=4) as p2t,
        tc.tile_pool(name="p2ps", bufs=3, space="PSUM") as p2ps,
        tc.tile_pool(name="p2acc", bufs=2, space="PSUM") as p2acc,
        tc.tile_pool(name="p3y", bufs=6) as p3y,
        tc.tile_pool(name="p3ps", bufs=3, space="PSUM") as p3ps,
    ):
        for st in range(NST):
            nj = 4 * st + 4          # number of t-blocks
            s0 = STILE * st
            for h in range(QH):
                ET = p2et.tile([128, NS, STILE], BF16, tag="et")
                acc = p2t.tile([128, STILE], F32R, tag="acc")
                ps_av = p2acc.tile([128, STILE], F32, tag="av")
                for j in range(nj):
                    k = j - (nj - 4)
                    # diagonal blocks: shrink to the exact causal span
                    m = max(k, 0)
                    off = 128 * m
                    wid = STILE - off
                    ps_st = p2ps.tile([128, STILE], F32, tag="st",
                                      name=f"pst{st}_{h}_{j}")
                    nc.tensor.matmul(
                        ps_st[:, 0:wid],
                        lhsT=QT_all[:, QH, 128 * j:128 * (j + 1)],
                        rhs=QT_all[:, h, s0 + off:s0 + STILE],
                        start=True, stop=True,
                    )
                    nc.scalar.activation(
                        ET[:, j, 0:wid], ps_st[:, 0:wid], AF.Exp, scale=SCALE)
                    if k >= 0:
                        # zero the invalid upper triangle of the diagonal
                        # 128-block after exp (scores are bounded, so exp of
                        # unmasked junk is finite). Pool engine on SBUF, off
                        # both the exp and the DVE denominator paths.
                        nc.gpsimd.tensor_mul(
                            ET[:, j, 0:128], ET[:, j, 0:128], tri_mask)
                    nc.tensor.matmul(
                        ps_av[:, off:STILE], lhsT=V_all[:, j, :],
                        rhs=ET[:, j, 0:wid],
                        start=(j == 0), stop=(j == nj - 1),
                        skip_group_check=True,
                    )
                    # denominator: running elementwise sum over t-blocks on
                    # DVE (fast enough to track the exp cadence)
                    if j == 0:
                        nc.vector.tensor_copy(acc, ET[:, 0, :])
                    else:
                        nc.vector.tensor_add(
                            acc[:, off:STILE], acc[:, off:STILE],
                            ET[:, j, 0:wid])
                # single cross-partition reduce for the denominator
                ps_den = p2ps.tile([128, STILE], F32, tag="st",
                                   name=f"den{st}_{h}")
                nc.tensor.matmul(ps_den, lhsT=ones_r, rhs=acc,
                                 start=True, stop=True)
                den_r = p2t.tile([128, STILE], F32, tag="denr")
                nc.vector.reciprocal_approx_fast(den_r, ps_den)
                ot_f = p2t.tile([128, STILE], F32, tag="otf")
                nc.vector.tensor_mul(ot_f, ps_av, den_r)
                nc.vector.tensor_copy(OT_all[:, h, s0:s0 + STILE], ot_f)

            # ---------- phase 3 for this s-tile ----------
            for dc in range(NDC):
                for sbl in range(4):
                    sb = 4 * st + sbl
                    ps_y = p3ps.tile([128, 512], F32, tag="psy",
                                     name=f"psy{st}_{dc}_{sbl}")
                    for ob in range(QH):
                        nc.tensor.matmul(
                            ps_y,
                            lhsT=OT_all[:, ob, 128 * sb:128 * (sb + 1)],
                            rhs=wo_sb[:, dc, ob, :],
                            start=(ob == 0), stop=(ob == QH - 1),
                        )
                    y_sb = p3y.tile([128, 512], BF16, tag="ysb")
                    if sb % 2 == 0:
                        nc.vector.tensor_copy(y_sb, ps_y)
                    else:
                        nc.scalar.copy(y_sb, ps_y)
                    nc.sync.dma_start(
                        y[128 * sb:128 * (sb + 1), 512 * dc:512 * (dc + 1)],
                        y_sb)


_NC_CACHE = None


def _get_nc():
    global _NC_CACHE
    if _NC_CACHE is None:
        _NC_CACHE = _build_nc()
    return _NC_CACHE


def _prep_in_maps(x, freqs_cos, freqs_sin, wqkv, wo):
    bf = ml_dtypes.bfloat16
    xT = x.reshape(S, DIM).T.astype(bf)                        # [DIM, S]
    # xt[sb, xs, p, n, s] = xT[128*(XSUB*xs+n)+p, 128*sb+s]
    xt = np.ascontiguousarray(
        xT.reshape(NXS, XSUB, 128, NS, 128).transpose(3, 0, 2, 1, 4))
    cos5 = np.ascontiguousarray(np.tile(freqs_cos, (1, 5)))    # [S, 320]
    sin5 = np.ascontiguousarray(np.tile(freqs_sin, (1, 5)))

    # multiplicative 0/1 triangle mask for the diagonal 128x128 sub-block:
    # row t', col c valid iff c >= t'
    tl = np.arange(128)[:, None]
    cl = np.arange(128)[None, :]
    cm = np.where(cl >= tl, 1.0, 0.0).astype(bf)
    cm = np.ascontiguousarray(cm)

    in_maps = []
    for i in range(N_CORES):
        wq = wqkv[QH * HD * i: QH * HD * (i + 1)]               # [512, DIM]
        wk = wqkv[N_HEADS * HD + HD * i: N_HEADS * HD + HD * (i + 1)]
        wv = wqkv[N_HEADS * HD + N_KV_HEADS * HD + HD * i:
                  N_HEADS * HD + N_KV_HEADS * HD + HD * (i + 1)]
        wT = np.concatenate([wq, wk, wv], axis=0).T.astype(bf)  # [DIM, 768]
        # wt[p, db, o] = wT[128*db+p, o]
        wt = np.ascontiguousarray(wT.reshape(ND, 128, OC).transpose(1, 0, 2))
        woT = wo[:, QH * HD * i: QH * HD * (i + 1)].T.astype(bf)  # [512, DIM]
        # wot[p, dc, ob, j] = woT[128*ob+p, 512*dc+j]
        wot = np.ascontiguousarray(
            woT.reshape(QH, 128, NDC, 512).transpose(1, 2, 0, 3))
        in_maps.append({
            "xt": xt, "wt": wt, "wot": wot,
            "cos5": cos5, "sin5": sin5, "cmask": cm,
        })
    return in_maps


def kernel(x, freqs_cos, freqs_sin, mask, wqkv, wo, _want_trace=False):
    x = np.asarray(x, np.float32)
    freqs_cos = np.asarray(freqs_cos, np.float32)
    freqs_sin = np.asarray(freqs_sin, np.float32)
    wqkv = np.asarray(wqkv, np.float32)
    wo = np.asarray(wo, np.float32)

    nc = _get_nc()
    in_maps = _prep_in_maps(x, freqs_cos, freqs_sin, wqkv, wo)
    res = run_bass_kernel_spmd(
        nc, in_maps, core_ids=list(range(N_CORES)), trace=_want_trace,
    )
    out = np.zeros((S, DIM), np.float64)
    for r in res.results:
        out += r["y"].astype(np.float64)
    if _want_trace:
        kernel._last_results = res
    return out.astype(np.float32).reshape(B, S, DIM)


# revision 16
# speedup vs baseline: 1.3161x; 1.1541x over previous
"""Trainium2 Bass kernel for GQA attention block (B=1, S=2048, DIM=4096,
32 q heads / 8 kv heads, head_dim 128, RoPE, causal, fused QKV + out proj).

Sharding: tensor-parallel over heads across 8 cores. Core i computes
q heads 4i..4i+3 and kv head i (one full GQA group), plus the wo
contribution of its 512 output columns; host sums the 8 partial outputs.

All heavy matmuls run in bf16 (rel err vs fp32 reference ~3.5e-3 <<
2e-2 gate). The attention inner loop is exp-cadence-bound on the
Scalar engine, so output-projection matmul chains are interleaved into
the attention j-loop via a filler queue to keep the PE saturated
(which also keeps its DVFS clock up).
"""
import numpy as np
import ml_dtypes

import concourse.bass as bass
import concourse.mybir as mybir
import concourse.tile as tile
from concourse import bacc
from concourse.bass_utils import run_bass_kernel_spmd
from concourse.masks import make_identity

F32 = mybir.dt.float32
F32R = mybir.dt.float32r
BF16 = mybir.dt.bfloat16
AF = mybir.ActivationFunctionType

B, S, DIM = 1, 2048, 4096
N_HEADS, N_KV_HEADS = 32, 8
HD = DIM // N_HEADS              # 128
N_CORES = 8
QH = N_HEADS // N_CORES          # 4 q heads per core
OC = QH * HD + 2 * HD            # 768 per-core qkv output columns
NS = S // 128                    # 16 s-blocks
ND = DIM // 128                  # 32 d-blocks
XSUB = 8                         # d-blocks per x sub-tile in phase 1
NXS = ND // XSUB                 # 4 x sub-tiles per s-block
WSUB = 4                         # d-blocks per w load chunk
STILE = 512                      # s-tile width in phase 2/3
NST = S // STILE                 # 4 s-tiles
NDC = DIM // 512                 # 8 output column chunks
SCALE = 1.0 / float(np.sqrt(HD))


def _build_nc():
    nc = bacc.Bacc("TRN2", target_bir_lowering=False, debug=False)

    # host-pre-tiled inputs (see _prep_in_maps for layouts)
    xt = nc.dram_tensor("xt", [NS, NXS, 128, XSUB, 128], BF16,
                        kind="ExternalInput").ap()
    wt = nc.dram_tensor("wt", [128, ND, OC], BF16, kind="ExternalInput").ap()
    wot = nc.dram_tensor("wot", [128, NDC, QH, 512], BF16,
                         kind="ExternalInput").ap()
    cos5 = nc.dram_tensor("cos5", [S, 5 * 64], F32, kind="ExternalInput").ap()
    sin5 = nc.dram_tensor("sin5", [S, 5 * 64], F32, kind="ExternalInput").ap()
    # [128, 128] multiplicative 0/1 triangle mask for the diagonal 128-block
    cmask = nc.dram_tensor("cmask", [128, 128], BF16, kind="ExternalInput").ap()
    y = nc.dram_tensor("y", [S, DIM], BF16, kind="ExternalOutput").ap()

    with tile.TileContext(nc) as tc:
        _emit(tc, nc, xt, wt, wot, cos5, sin5, cmask, y)
    nc.compile()
    return nc


def _emit(tc, nc, xt, wt, wot, cos5, sin5, cmask, y):
    import contextlib

    with contextlib.ExitStack() as ctx:
        # ---------- long-lived tiles ----------
        keep = ctx.enter_context(tc.tile_pool(name="keep", bufs=1))
        # QT_all[:, h, :]: per-head roped Q transposed [d, s]; h=QH is roped K
        QT_all = keep.tile([128, QH + 1, S], BF16)
        V_all = keep.tile([128, NS, HD], BF16)          # V blocks [t, d]
        OT_all = keep.tile([128, QH, S], BF16)          # attn out transposed
        wo_sb = keep.tile([128, NDC, QH, 512], BF16)    # full wo slice (4MB)
        ones_f = keep.tile([128, 128], F32)
        nc.vector.memset(ones_f, 1.0)
        ones_r = keep.tile([128, 128], F32R)
        nc.vector.tensor_copy(ones_r, ones_f)
        ident = keep.tile([128, 128], BF16)
        make_identity(nc, ident)
        tri_mask = keep.tile([128, 128], BF16)
        nc.gpsimd.dma_start(tri_mask, cmask)

        # ---------- phase 1: qkv projection + RoPE + transposes ----------
        with (
            tc.tile_pool(name="p1w", bufs=1) as p1w,
            tc.tile_pool(name="p1x", bufs=2) as p1x,
            tc.tile_pool(name="p1t", bufs=2) as p1t,
            tc.tile_pool(name="p1ps", bufs=2, space="PSUM") as p1ps,
        ):
            # first x sub-tile before the w bulk so PE can start ASAP
            x_first = p1x.tile([128, XSUB, 128], BF16, tag="x")
            nc.scalar.dma_start(x_first, xt[0, 0])
            # w chunked so the first matmuls can start after the first chunk
            w_sb = p1w.tile([128, ND, OC], BF16)
            for ci in range(ND // WSUB):
                nc.sync.dma_start(
                    w_sb[:, WSUB * ci:WSUB * (ci + 1), :],
                    wt[:, WSUB * ci:WSUB * (ci + 1), :],
                )
            # prefetch the whole wo slice during phase 1 (sync queue drains
            # after the w chunks; transfer hides under phase-1 compute)
            nc.sync.dma_start(wo_sb, wot)

            # s-blocks in groups of GRP; double-buffered PSUM accumulators,
            # with rope/transpose of group g deferred until after group g+1's
            # matmuls so the PE never waits on the DVE rope chain.
            GRP = 2
            groups = [list(range(g, min(g + GRP, NS))) for g in range(0, NS, GRP)]
            pend_rope = None
            for group in groups:
                ps_qs = {}
                ps_kvs = {}
                x_tiles = {}
                for sb in group:
                    ps_qs[sb] = p1ps.tile([128, 512], F32, tag=f"psq{sb % GRP}", name=f"psq{sb}")
                    ps_kvs[sb] = p1ps.tile([128, 256], F32, tag=f"pskv{sb % GRP}", name=f"pskv{sb}")
                for xs in range(NXS):
                    for sb in group:
                        if sb == 0 and xs == 0:
                            x_tiles[sb] = x_first
                        else:
                            x_tiles[sb] = p1x.tile(
                                [128, XSUB, 128], BF16, tag=f"x{sb % GRP}",
                                name=f"x{sb}_{xs}")
                            nc.scalar.dma_start(x_tiles[sb], xt[sb, xs])
                    for sb in group:
                        x_sb = x_tiles[sb]
                        for dbi in range(XSUB):
                            db = XSUB * xs + dbi
                            nc.tensor.matmul(
                                ps_qs[sb], lhsT=x_sb[:, dbi, :],
                                rhs=w_sb[:, db, 0:512],
                                start=(db == 0), stop=(db == ND - 1),
                            )
                            nc.tensor.matmul(
                                ps_kvs[sb], lhsT=x_sb[:, dbi, :],
                                rhs=w_sb[:, db, 512:768],
                                start=(db == 0), stop=(db == ND - 1),
                            )
                if pend_rope is not None:
                    for sb, pq, pkv in pend_rope:
                        _rope_and_transpose(
                            tc, nc, p1t, p1ps, cos5, sin5, sb,
                            pq, pkv, QT_all, V_all, ident)
                pend_rope = [(sb, ps_qs[sb], ps_kvs[sb]) for sb in group]
            for sb, pq, pkv in pend_rope:
                _rope_and_transpose(
                    tc, nc, p1t, p1ps, cos5, sin5, sb,
                    pq, pkv, QT_all, V_all, ident)

        _emit_attn(tc, nc, ctx, (QT_all, V_all, OT_all, wo_sb, ones_r,
                                 tri_mask), y)


def _rope_and_transpose(tc, nc, p1t, p1ps, cos5, sin5, sb, ps_q, ps_kv,
                        QT_all, V_all, ident):
    # RoPE (q: 4 heads = 512 cols; k: 128 cols), f32 math
    cos_t = p1t.tile([128, 320], F32, tag="cos")
    sin_t = p1t.tile([128, 320], F32, tag="sin")
    nc.gpsimd.dma_start(cos_t, cos5[128 * sb:128 * (sb + 1), :])
    nc.gpsimd.dma_start(sin_t, sin5[128 * sb:128 * (sb + 1), :])

    qk_roped = p1t.tile([128, 640], F32, tag="qkr")
    for part, ps_src, wid in (("q", ps_q, 512), ("k", ps_kv, 128)):
        nf = wid // 2
        off = 0 if part == "q" else 512
        pe = ps_src[:, 0:wid:2]
        po = ps_src[:, 1:wid:2]
        c = cos_t[:, 0:nf]
        sn = sin_t[:, 0:nf]
        t1 = p1t.tile([128, 256], F32, tag="t1")
        t2 = p1t.tile([128, 256], F32, tag="t2")
        nc.vector.tensor_mul(t1[:, 0:nf], pe, c)
        nc.vector.tensor_mul(t2[:, 0:nf], po, sn)
        nc.vector.tensor_sub(
            qk_roped[:, off + 0:off + wid:2], t1[:, 0:nf], t2[:, 0:nf])
        t3 = p1t.tile([128, 256], F32, tag="t3")
        t4 = p1t.tile([128, 256], F32, tag="t4")
        nc.vector.tensor_mul(t3[:, 0:nf], pe, sn)
        nc.vector.tensor_mul(t4[:, 0:nf], po, c)
        nc.vector.tensor_add(
            qk_roped[:, off + 1:off + wid:2], t3[:, 0:nf], t4[:, 0:nf])

    # bf16 copy of the roped q/k for the PE transposes
    qk_bf = p1t.tile([128, 640], BF16, tag="qkb")
    nc.vector.tensor_copy(qk_bf, qk_roped)

    # V block: natural [t, d] (cast f32 psum -> bf16)
    nc.vector.tensor_copy(V_all[:, sb, :], ps_kv[:, 128:256])

    # transpose roped q/k head-slices into QT_all
    for h in range(QH + 1):
        # borrow q/kv accumulator slots (pool-tag reuse; tile's WAR
        # tracking orders this after the rope/V reads)
        tag = f"psq{sb % 2}" if h % 2 == 0 else f"pskv{sb % 2}"
        ps_t = p1ps.tile([128, 128], BF16, tag=tag, name=f"pst{sb}_{h}")
        nc.tensor.transpose(ps_t, qk_bf[:, 128 * h:128 * (h + 1)], ident)
        nc.vector.tensor_copy(QT_all[:, h, 128 * sb:128 * (sb + 1)], ps_t)


def _emit_attn(tc, nc, ctx, keep_tiles, y):
    (QT_all, V_all, OT_all, wo_sb, ones_r, tri_mask) = keep_tiles
    # ---------- phases 2+3, software-pipelined ----------
    with (
        tc.tile_pool(name="p2et", bufs=2) as p2et,
        tc.tile_pool(name="p2t", bufs=4) as p2t,
        tc.tile_pool(name="p2ps", bufs=3, space="PSUM") as p2ps,
        tc.tile_pool(name="p2acc", bufs=2, space="PSUM") as p2acc,
        tc.tile_pool(name="p3y", bufs=6) as p3y,
        tc.tile_pool(name="p3ps", bufs=3, space="PSUM") as p3ps,
    ):
        filler = []          # pending output-projection chunk closures
        chunk_no = [0]

        def emit_chunk(st, dc, sbl):
            sb = 4 * st + sbl
            ps_y = p3ps.tile([128, 512], F32, tag="psy",
                             name=f"psy{st}_{dc}_{sbl}")
            for ob in range(QH):
                nc.tensor.matmul(
                    ps_y,
                    lhsT=OT_all[:, ob, 128 * sb:128 * (sb + 1)],
                    rhs=wo_sb[:, dc, ob, :],
                    start=(ob == 0), stop=(ob == QH - 1),
                )
            y_sb = p3y.tile([128, 512], BF16, tag="ysb")
            chunk_no[0] += 1
            if chunk_no[0] % 2 == 0:
                nc.vector.tensor_copy(y_sb, ps_y)
            else:
                nc.scalar.copy(y_sb, ps_y)
            nc.sync.dma_start(
                y[128 * sb:128 * (sb + 1), 512 * dc:512 * (dc + 1)], y_sb)

        def pop_filler():
            if filler:
                filler.pop(0)()

        def main_block(st, h):
            nj = 4 * st + 4
            s0 = STILE * st
            ET = p2et.tile([128, NS, STILE], BF16, tag="et",
                           name=f"et{st}_{h}")
            acc = p2t.tile([128, STILE], F32R, tag="acc", name=f"acc{st}_{h}")
            ps_av = p2acc.tile([128, STILE], F32, tag="av",
                               name=f"av{st}_{h}")
            for j in range(nj):
                k = j - (nj - 4)
                m = max(k, 0)
                off = 128 * m
                wid = STILE - off
                ps_st = p2ps.tile([128, STILE], F32, tag="st",
                                  name=f"pst{st}_{h}_{j}")
                nc.tensor.matmul(
                    ps_st[:, 0:wid],
                    lhsT=QT_all[:, QH, 128 * j:128 * (j + 1)],
                    rhs=QT_all[:, h, s0 + off:s0 + STILE],
                    start=True, stop=True,
                )
                # independent output-projection work to fill the PE while
                # the exp for this j runs on the Scalar engine
                pop_filler()
                nc.scalar.activation(
                    ET[:, j, 0:wid], ps_st[:, 0:wid], AF.Exp, scale=SCALE)
                if k >= 0:
                    # zero the invalid upper triangle of the diagonal
                    # 128-block after exp (scores are bounded, so exp of
                    # unmasked junk is finite). Pool engine on SBUF.
                    nc.gpsimd.tensor_mul(
                        ET[:, j, 0:128], ET[:, j, 0:128], tri_mask)
                nc.tensor.matmul(
                    ps_av[:, off:STILE], lhsT=V_all[:, j, :],
                    rhs=ET[:, j, 0:wid],
                    start=(j == 0), stop=(j == nj - 1),
                    skip_group_check=True,
                )
                # denominator: running elementwise sum over t-blocks on DVE
                if j == 0:
                    nc.vector.tensor_copy(acc, ET[:, 0, :])
                else:
                    nc.vector.tensor_add(
                        acc[:, off:STILE], acc[:, off:STILE],
                        ET[:, j, 0:wid])
            return acc, ps_av

        def finalize(st, h, acc, ps_av):
            s0 = STILE * st
            ps_den = p2ps.tile([128, STILE], F32, tag="st",
                               name=f"den{st}_{h}")
            nc.tensor.matmul(ps_den, lhsT=ones_r, rhs=acc,
                             start=True, stop=True)
            den_r = p2t.tile([128, STILE], F32, tag="denr")
            nc.vector.reciprocal_approx_fast(den_r, ps_den)
            nc.vector.tensor_mul(OT_all[:, h, s0:s0 + STILE], ps_av, den_r)

        pending = None
        for st in range(NST):
            for h in range(QH):
                acc, ps_av = main_block(st, h)
                if pending is not None:
                    pst, ph, pacc, pav = pending
                    finalize(pst, ph, pacc, pav)
                    if ph == QH - 1:
                        for dc in range(NDC):
                            for sbl in range(4):
                                filler.append(
                                    (lambda a, b, c: lambda: emit_chunk(a, b, c))(pst, dc, sbl))
                pending = (st, h, acc, ps_av)

        pst, ph, pacc, pav = pending
        finalize(pst, ph, pacc, pav)
        for dc in range(NDC):
            for sbl in range(4):
                filler.append(
                    (lambda a, b, c: lambda: emit_chunk(a, b, c))(pst, dc, sbl))
        while filler:
            pop_filler()


_NC_CACHE = None


def _get_nc():
    global _NC_CACHE
    if _NC_CACHE is None:
        _NC_CACHE = _build_nc()
    return _NC_CACHE


def _prep_in_maps(x, freqs_cos, freqs_sin, wqkv, wo):
    bf = ml_dtypes.bfloat16
    xT = x.reshape(S, DIM).T.astype(bf)                        # [DIM, S]
    # xt[sb, xs, p, n, s] = xT[128*(XSUB*xs+n)+p, 128*sb+s]
    xt = np.ascontiguousarray(
        xT.reshape(NXS, XSUB, 128, NS, 128).transpose(3, 0, 2, 1, 4))
    cos5 = np.ascontiguousarray(np.tile(freqs_cos, (1, 5)))    # [S, 320]
    sin5 = np.ascontiguousarray(np.tile(freqs_sin, (1, 5)))

    # multiplicative 0/1 triangle mask for the diagonal 128x128 sub-block:
    # row t', col c valid iff c >= t'
    tl = np.arange(128)[:, None]
    cl = np.arange(128)[None, :]
    cm = np.where(cl >= tl, 1.0, 0.0).astype(bf)
    cm = np.ascontiguousarray(cm)

    in_maps = []
    for i in range(N_CORES):
        wq = wqkv[QH * HD * i: QH * HD * (i + 1)]               # [512, DIM]
        wk = wqkv[N_HEADS * HD + HD * i: N_HEADS * HD + HD * (i + 1)]
        wv = wqkv[N_HEADS * HD + N_KV_HEADS * HD + HD * i:
                  N_HEADS * HD + N_KV_HEADS * HD + HD * (i + 1)]
        wT = np.concatenate([wq, wk, wv], axis=0).T.astype(bf)  # [DIM, 768]
        # wt[p, db, o] = wT[128*db+p, o]
        wt = np.ascontiguousarray(wT.reshape(ND, 128, OC).transpose(1, 0, 2))
        woT = wo[:, QH * HD * i: QH * HD * (i + 1)].T.astype(bf)  # [512, DIM]
        # wot[p, dc, ob, j] = woT[128*ob+p, 512*dc+j]
        wot = np.ascontiguousarray(
            woT.reshape(QH, 128, NDC, 512).transpose(1, 2, 0, 3))
        in_maps.append({
            "xt": xt, "wt": wt, "wot": wot,
            "cos5": cos5, "sin5": sin5, "cmask": cm,
        })
    return in_maps


def kernel(x, freqs_cos, freqs_sin, mask, wqkv, wo, _want_trace=False):
    x = np.asarray(x, np.float32)
    freqs_cos = np.asarray(freqs_cos, np.float32)
    freqs_sin = np.asarray(freqs_sin, np.float32)
    wqkv = np.asarray(wqkv, np.float32)
    wo = np.asarray(wo, np.float32)

    nc = _get_nc()
    in_maps = _prep_in_maps(x, freqs_cos, freqs_sin, wqkv, wo)
    res = run_bass_kernel_spmd(
        nc, in_maps, core_ids=list(range(N_CORES)), trace=_want_trace,
    )
    out = np.zeros((S, DIM), np.float64)
    for r in res.results:
        out += r["y"].astype(np.float64)
    if _want_trace:
        kernel._last_results = res
    return out.astype(np.float32).reshape(B, S, DIM)
